# revision 1
# baseline (speedup 1.0000x reference)
"""Dense SE(3) Gauss-Newton kernel for Trainium2, sharded over 8 NeuronCores.

Sharding: core owns batch b = core//4 and a 256-anchor slab of the i axis;
the k axis (1024) runs in 8 chunks of 128 on the partition dimension with
anchors on the free dimension. Per (k,i) quantities reduce over k on the
TensorEngine (float32r single-pass matmuls) against per-k polynomial columns;
the 6x6 normal-equation assembly is itself a matmul against a constant 0/1
combination matrix, then a PE transpose puts anchors back on partitions for
the solve / exp-map / compose stage.
"""
import sys

sys.path.insert(0, "/opt/trn_rl_repo")

import numpy as np

from concourse import bacc, tile, masks
import concourse.mybir as mybir
import concourse.bass_utils as _bu
from concourse.bass_utils import run_bass_kernel_spmd

# Re-enable walrus LDWEIGHTS dedup: consecutive reduction matmuls here share
# one stationary operand, and the per-matmul reload is pure overhead.
if not getattr(_bu, "_ldw_patch", False):
    _orig_run_command = _bu.run_command

    def _run_command_ldw(cmd, *a, **kw):
        cmd = ["--enable-ldw-opt=true" if c == "--enable-ldw-opt=false" else c
               for c in cmd]
        return _orig_run_command(cmd, *a, **kw)

    _bu.run_command = _run_command_ldw
    _bu._ldw_patch = True

F32 = mybir.dt.float32
F32R = mybir.dt.float32r
AF = mybir.ActivationFunctionType
ALU = mybir.AluOpType
AX = mybir.AxisListType

B, C, H, W = 2, 16, 32, 32
N = H * W
NCORES = 8
SLAB = 256
KC = 8
P = 128

# poly columns for the Hm/rhs reduction matmuls
# t:   0  1  2  3  4    5    6    7   8   9   10  11  12  13   14   15   16   17   18
#     [1, x, y, z, x2,  y2,  z2,  xy, xz, yz, -x, -y, -z, -xy, -xz, -yz, 2yz, 2xz, -1]
NT = 19
# acc band order m: 0:M00 1:M11 2:M02p 3:M12p 4:M22 5:g0 6:g1 7:g2n
# band m lives in acc tile m//4 at partitions 0:19, cols (m%4)*SLAB .. +SLAB
NM = 8

# (row, col) of the 6x7 augmented system -> summed acc (m, t) terms
HM_TABLE = {
    (0, 0): [(0, 0)],
    (0, 2): [(2, 18)],
    (0, 3): [(2, 2)],
    (0, 4): [(0, 12), (2, 10)],
    (0, 5): [(0, 2)],
    (0, 6): [(5, 0)],
    (1, 1): [(1, 0)],
    (1, 2): [(3, 18)],
    (1, 3): [(1, 3), (3, 2)],
    (1, 4): [(3, 10)],
    (1, 5): [(1, 10)],
    (1, 6): [(6, 0)],
    (2, 2): [(4, 0)],
    (2, 3): [(3, 12), (4, 11)],
    (2, 4): [(2, 3), (4, 1)],
    (2, 5): [(2, 11), (3, 1)],
    (2, 6): [(7, 18)],
    (3, 3): [(1, 6), (3, 16), (4, 5)],
    (3, 4): [(3, 14), (2, 15), (4, 13)],
    (3, 5): [(1, 14), (2, 5), (3, 13)],
    (3, 6): [(6, 3), (7, 2)],
    (4, 4): [(0, 6), (2, 17), (4, 4)],
    (4, 5): [(0, 15), (2, 13), (3, 4)],
    (4, 6): [(5, 12), (7, 10)],
    (5, 5): [(0, 5), (1, 4)],
    (5, 6): [(5, 2), (6, 10)],
}

NE = 48  # entry columns (6x7 augmented = 42 used, padded)


def combo_matrices():
    """0/1 matrix [NT, NM*NE] mapping acc rows -> augmented-system entries."""
    cm = np.zeros((NT, NM * NE), np.float32)
    for (r, cc), terms in HM_TABLE.items():
        es = [r * 7 + cc]
        if cc < 6 and r != cc:
            es.append(cc * 7 + r)
        for m, t in terms:
            for e in es:
                cm[t, m * NE + e] = 1.0
    return cm


def build_nc():
    nc = bacc.Bacc("TRN2", target_bir_lowering=False, debug=False)

    def din(name, shape):
        return nc.dram_tensor(name, list(shape), F32, kind="ExternalInput")

    # kblob cols: zk 0:8 | uk 8:16 | vk 16:24 | tm_k 24:152 | rev_k 152:176 | w_k 176:200
    kblob_d = din("kblob", (P, 200))
    ek_d = din("ek", (C, N))
    uvzz_d = din("uvzz", (2, N))
    zz2_d = din("zz2", (2, N))
    zo_d = din("zo", (2, N))
    ei_d = din("ei", (C, SLAB))
    cam_d = din("cam", (1, 4))
    cam2_d = din("cam2", (2, 2))
    rhsj_d = din("rhsj", (21, 4 * SLAB))
    cmbt_d = din("cmbt", (NT, NM * NE))
    tmi_d = din("tm_i", (SLAB, 16))
    out_d = nc.dram_tensor("out", [SLAB, 16], F32, kind="ExternalOutput")

    with tile.TileContext(nc) as tc:
        with tc.tile_pool(name="persist", bufs=1) as pp, \
             tc.tile_pool(name="acc_ps", bufs=1, space="PSUM") as accp:

            # ---------------- inputs -> SBUF ----------------
            kblob = pp.tile([P, 200], F32)
            ek = pp.tile([C, N], F32)
            uvzz = pp.tile([2, N], F32)
            zz2 = pp.tile([2, N], F32)
            ei = pp.tile([C, SLAB], F32)
            cam = pp.tile([1, 4], F32)
            zk = kblob[:, 0:8]
            uk = kblob[:, 8:16]
            vk = kblob[:, 16:24]
            tmk = kblob[:, 24:152]
            revk = kblob[:, 152:176]
            wk = kblob[:, 176:200]
            cam2 = pp.tile([2, 2], F32)
            rhsj_s = pp.tile([21, 4 * SLAB], F32)
            cmbt_s = pp.tile([NT, NM * NE], F32)
            rhsj = pp.tile([21, 4 * SLAB], F32R)
            cmbt = pp.tile([NT, NM * NE], F32R)
            tmi0 = pp.tile([P, 16], F32)
            tmi1 = pp.tile([P, 16], F32)
            for t, d in [(kblob, kblob_d), (ek, ek_d), (uvzz, uvzz_d),
                         (zz2, zz2_d), (ei, ei_d), (cam, cam_d),
                         (cam2, cam2_d), (rhsj_s, rhsj_d), (cmbt_s, cmbt_d)]:
                nc.sync.dma_start(t[:], d[:])
            nc.sync.dma_start(tmi0[:], tmi_d[0:P, :])
            nc.sync.dma_start(tmi1[:], tmi_d[P : 2 * P, :])

            ones1 = pp.tile([1, P], F32)
            ones16 = pp.tile([C, 1], F32)
            nc.vector.memset(ones1[:], 1.0)
            nc.vector.memset(ones16[:], 1.0)
            ident = pp.tile([P, P], F32)
            masks.make_identity(nc, ident[:])

            # persistent accumulators: tile j holds bands m=4j..4j+3 as
            # side-by-side column blocks (one start per 2KB psum bank)
            acc_ps0 = accp.tile([32, 4 * SLAB], F32)
            acc_ps1 = accp.tile([32, 4 * SLAB], F32)
            acc_ps = [acc_ps0, acc_ps1]

            # ---------------- setup ----------------
            with tc.tile_pool(name="setup_ps", bufs=1, space="PSUM") as sps:
                cam_psb = sps.tile([P, 4], F32)
                nc.tensor.matmul(cam_psb[:], ones1[:], cam[:], start=True, stop=True)
                camb = pp.tile([P, 4], F32)
                nc.scalar.copy(camb[:], cam_psb[:])

                eisq = pp.tile([C, SLAB], F32)
                nc.scalar.square(eisq[:], ei[:])
                ei2r_ps = sps.tile([1, SLAB], F32)
                nc.tensor.matmul(ei2r_ps[:], ones16[:], eisq[:], start=True, stop=True)
                ei2r = pp.tile([1, SLAB], F32)
                nc.scalar.copy(ei2r[:], ei2r_ps[:])

                eksq = pp.tile([C, N], F32)
                nc.scalar.square(eksq[:], ek[:])
                ek2r_ps = sps.tile([1, N], F32)
                nc.tensor.matmul(ek2r_ps[:, 0 : N // 2], ones16[:],
                                 eksq[:, 0 : N // 2], start=True, stop=True)
                nc.tensor.matmul(ek2r_ps[:, N // 2 : N], ones16[:],
                                 eksq[:, N // 2 : N], start=True, stop=True)
                ek2r = pp.tile([1, N], F32)
                nc.scalar.copy(ek2r[:], ek2r_ps[:])

            nc.sync.dma_start(rhsj_s[19:20, 3 * SLAB : 4 * SLAB], ei2r[:])
            nc.scalar.copy(rhsj[:], rhsj_s[:])
            nc.scalar.copy(cmbt[:], cmbt_s[:])
            rhsjl = pp.tile([21, 4 * SLAB], F32R)
            nc.vector.tensor_tensor(rhsjl[:], rhsj_s[:], rhsj[:].bitcast(F32),
                                    ALU.subtract)

            invfx = pp.tile([P, 1], F32)
            invfy = pp.tile([P, 1], F32)
            nc.vector.reciprocal(invfx[:], camb[:, 0:1])
            nc.vector.reciprocal(invfy[:], camb[:, 2:3])
            negf = pp.tile([P, 2], F32)
            nc.vector.tensor_scalar(negf[:], camb[:, 0:4:2], -1.0, None, ALU.mult)
            f2 = pp.tile([P, 2], F32)  # fx^2, fy^2
            nc.vector.tensor_tensor(f2[:], camb[:, 0:4:2], camb[:, 0:4:2], ALU.mult)
            # weights pre-scaled: w0*fx^2, w1*fy^2 ([128, KC] each)
            wf0 = pp.tile([P, KC], F32)
            wf1 = pp.tile([P, KC], F32)
            nc.vector.tensor_scalar(wf0[:], wk[:, 0 : KC * 3 : 3], f2[:, 0:1], None, ALU.mult)
            nc.vector.tensor_scalar(wf1[:], wk[:, 1 : KC * 3 : 3], f2[:, 1:2], None, ALU.mult)

            # k-major pointcloud x, y
            xk = pp.tile([P, KC], F32)
            yk = pp.tile([P, KC], F32)
            tmpk = pp.tile([P, KC], F32)
            nc.vector.tensor_scalar(tmpk[:], uk[:], camb[:, 1:2], None, ALU.subtract)
            nc.vector.tensor_tensor(tmpk[:], tmpk[:], zk[:], ALU.mult)
            nc.vector.tensor_scalar(xk[:], tmpk[:], invfx[:], None, ALU.mult)
            nc.vector.tensor_scalar(tmpk[:], vk[:], camb[:, 3:4], None, ALU.subtract)
            nc.vector.tensor_tensor(tmpk[:], tmpk[:], zk[:], ALU.mult)
            nc.vector.tensor_scalar(yk[:], tmpk[:], invfy[:], None, ALU.mult)

            # joint stationary [21, N]: rows -2e (16), x, y, z, 1, ek2
            xyzTs = pp.tile([21, N], F32)
            xyzTj = pp.tile([21, N], F32R)
            invf2 = pp.tile([2, 1], F32)
            nc.vector.reciprocal(invf2[:], cam2[:, 1:2])
            stg = pp.tile([2, N], F32)
            nc.vector.tensor_scalar(stg[:], uvzz[:], cam2[:, 0:1], None, ALU.subtract)
            nc.vector.tensor_tensor(stg[:], stg[:], zz2[:], ALU.mult)
            nc.vector.tensor_scalar(stg[:], stg[:], invf2[:], None, ALU.mult)
            nc.scalar.mul(xyzTs[0:C, :], ek[:], -2.0)
            nc.sync.dma_start(xyzTs[C : C + 2, :], stg[:])
            nc.sync.dma_start(xyzTs[C + 2 : C + 4, :], zo_d[:])
            nc.sync.dma_start(xyzTs[C + 4 : C + 5, :], ek2r[:])
            nc.scalar.copy(xyzTj[:], xyzTs[:])
            xyzTl = pp.tile([21, N], F32R)
            nc.vector.tensor_tensor(xyzTl[:], xyzTs[:], xyzTj[:].bitcast(F32),
                                    ALU.subtract)

            # poly columns [128, KC*NT]
            pol_s = pp.tile([P, KC * NT], F32)
            pol = pp.tile([P, KC * NT], F32R)
            E = KC * NT
            def pcol(t):
                return pol_s[:, t:E:NT]
            nc.vector.memset(pcol(0), 1.0)
            nc.vector.memset(pcol(18), -1.0)
            nc.vector.tensor_copy(pcol(1), xk[:])
            nc.vector.tensor_copy(pcol(2), yk[:])
            nc.vector.tensor_copy(pcol(3), zk[:])
            nc.vector.tensor_tensor(pcol(4), xk[:], xk[:], ALU.mult)
            nc.vector.tensor_tensor(pcol(5), yk[:], yk[:], ALU.mult)
            nc.vector.tensor_tensor(pcol(6), zk[:], zk[:], ALU.mult)
            nc.vector.tensor_tensor(pcol(7), xk[:], yk[:], ALU.mult)
            nc.vector.tensor_tensor(pcol(8), xk[:], zk[:], ALU.mult)
            nc.vector.tensor_tensor(pcol(9), yk[:], zk[:], ALU.mult)
            nc.vector.tensor_scalar(pcol(10), xk[:], -1.0, None, ALU.mult)
            nc.vector.tensor_scalar(pcol(11), yk[:], -1.0, None, ALU.mult)
            nc.vector.tensor_scalar(pcol(12), zk[:], -1.0, None, ALU.mult)
            nc.vector.tensor_scalar(pcol(13), pcol(7), -1.0, None, ALU.mult)
            nc.vector.tensor_scalar(pcol(14), pcol(8), -1.0, None, ALU.mult)
            nc.vector.tensor_scalar(pcol(15), pcol(9), -1.0, None, ALU.mult)
            nc.vector.tensor_scalar(pcol(16), pcol(9), 2.0, None, ALU.mult)
            nc.vector.tensor_scalar(pcol(17), pcol(8), 2.0, None, ALU.mult)
            nc.scalar.copy(pol[:], pol_s[:])

            # TjXj rows -> residual bias columns
            def tme(e):
                return tmk[:, e : KC * 16 : 16]
            tjx = [pp.tile([P, KC], F32, name=f"tjx{r}") for r in range(3)]
            sA = pp.tile([P, KC], F32)
            sB = pp.tile([P, KC], F32)
            for r in range(3):
                nc.vector.tensor_tensor(sA[:], tme(4 * r + 0), xk[:], ALU.mult)
                nc.vector.tensor_tensor(sB[:], tme(4 * r + 1), yk[:], ALU.mult)
                nc.vector.tensor_tensor(sA[:], sA[:], sB[:], ALU.add)
                nc.vector.tensor_tensor(sB[:], tme(4 * r + 2), zk[:], ALU.mult)
                nc.vector.tensor_tensor(sA[:], sA[:], sB[:], ALU.add)
                nc.vector.tensor_tensor(tjx[r][:], sA[:], tme(4 * r + 3), ALU.add)
            invzj = pp.tile([P, KC], F32)
            nc.vector.reciprocal(invzj[:], tjx[2][:])
            bias0 = pp.tile([P, KC], F32)
            bias1 = pp.tile([P, KC], F32)
            bias2 = pp.tile([P, KC], F32)
            nc.vector.tensor_tensor(sA[:], tjx[0][:], invzj[:], ALU.mult)
            nc.vector.tensor_scalar(sA[:], sA[:], negf[:, 0:1], None, ALU.mult)
            nc.vector.tensor_tensor(bias0[:], sA[:], revk[:, 0 : KC * 3 : 3], ALU.subtract)
            nc.vector.tensor_tensor(sA[:], tjx[1][:], invzj[:], ALU.mult)
            nc.vector.tensor_scalar(sA[:], sA[:], negf[:, 1:2], None, ALU.mult)
            nc.vector.tensor_tensor(bias1[:], sA[:], revk[:, 1 : KC * 3 : 3], ALU.subtract)
            nc.vector.tensor_scalar(sA[:], invzj[:], -1.0, None, ALU.mult)
            nc.vector.tensor_tensor(bias2[:], sA[:], revk[:, 2 : KC * 3 : 3], ALU.subtract)

            # stores that carry chunk results across the table-batched phases
            dall = pp.tile([P, KC * SLAB], F32)
            XpDall = pp.tile([P, KC * SLAB], F32)
            YpDall = pp.tile([P, KC * SLAB], F32)
            sall = pp.tile([P, KC * SLAB], F32)

            # -------- pipelined passes: A(half) -> batch(half) -> B(half) ----
            d2all = pp.tile([P, KC * SLAB], F32)
            d4all = pp.tile([P, KC * SLAB], F32)
            with tc.tile_pool(name="mm_ps", bufs=2, space="PSUM") as mmp, \
                 tc.tile_pool(name="work", bufs=2) as wp:

                def pass_a(c):
                    ck = slice(c * P, (c + 1) * P)
                    cs = slice(c * SLAB, (c + 1) * SLAB)
                    XY = mmp.tile([P, 2 * SLAB], F32, name=f"XY{c}", tag="XY")
                    ZD = mmp.tile([P, 2 * SLAB], F32, name=f"ZD{c}", tag="ZD")
                    lhs = xyzTj[:, ck]
                    lhsl = xyzTl[:, ck]
                    nc.tensor.matmul(XY[:], lhs, rhsj[:, 0 : 2 * SLAB],
                                     start=True, stop=False)
                    nc.tensor.matmul(XY[:], lhs, rhsjl[:, 0 : 2 * SLAB],
                                     start=False, stop=False)
                    nc.tensor.matmul(XY[:], lhsl, rhsj[:, 0 : 2 * SLAB],
                                     start=False, stop=True)
                    nc.tensor.matmul(ZD[:], lhs, rhsj[:, 2 * SLAB : 4 * SLAB],
                                     start=True, stop=False)
                    nc.tensor.matmul(ZD[:], lhs, rhsjl[:, 2 * SLAB : 4 * SLAB],
                                     start=False, stop=False)
                    nc.tensor.matmul(ZD[:], lhsl, rhsj[:, 2 * SLAB : 4 * SLAB],
                                     start=False, stop=True)
                    d = dall[:, cs]
                    nc.vector.reciprocal(d, ZD[:, 0:SLAB])
                    nc.vector.tensor_tensor(XpDall[:, cs], XY[:, 0:SLAB], d, ALU.mult)
                    nc.vector.tensor_tensor(YpDall[:, cs], XY[:, SLAB : 2 * SLAB], d, ALU.mult)
                    nc.scalar.copy(sall[:, cs], ZD[:, SLAB : 2 * SLAB])

                def batch(h):
                    hs = slice(h * 4 * SLAB, (h + 1) * 4 * SLAB)
                    nc.vector.tensor_scalar(sall[:, hs], sall[:, hs], 0.0, None, ALU.max)
                    nc.scalar.sqrt(sall[:, hs], sall[:, hs])
                    nc.scalar.activation(sall[:, hs], sall[:, hs], AF.Exp, scale=-1.0)
                    nc.vector.tensor_tensor(d2all[:, hs], dall[:, hs], dall[:, hs], ALU.mult)
                    nc.gpsimd.tensor_tensor(d4all[:, hs], d2all[:, hs], d2all[:, hs], ALU.mult)

                def pass_b(c):
                    cs = slice(c * SLAB, (c + 1) * SLAB)
                    d = dall[:, cs]
                    XpD = XpDall[:, cs]
                    YpD = YpDall[:, cs]
                    aff = sall[:, cs]
                    d2 = d2all[:, cs]
                    d4 = d4all[:, cs]
                    afd2 = wp.tile([P, SLAB], F32, name=f"afd2_{c}", tag="afd2")
                    w2pd2 = wp.tile([P, SLAB], F32, name=f"w2pd2_{c}", tag="w2pd2")
                    nc.vector.tensor_tensor(afd2[:], aff, d2, ALU.mult)
                    nc.scalar.mul(w2pd2[:], d2, wk[:, 3 * c + 2 : 3 * c + 3])

                    M00 = wp.tile([P, SLAB], F32R, name=f"M00_{c}", tag="M00")
                    M11 = wp.tile([P, SLAB], F32R, name=f"M11_{c}", tag="M11")
                    M02p = wp.tile([P, SLAB], F32R, name=f"M02p_{c}", tag="M02p")
                    M12p = wp.tile([P, SLAB], F32R, name=f"M12p_{c}", tag="M12p")
                    M22 = wp.tile([P, SLAB], F32R, name=f"M22_{c}", tag="M22")
                    t0 = wp.tile([P, SLAB], F32, name=f"t0_{c}", tag="t0")
                    t1 = wp.tile([P, SLAB], F32, name=f"t1_{c}", tag="t1")
                    t2 = wp.tile([P, SLAB], F32, name=f"t2_{c}", tag="t2")
                    nc.scalar.mul(M00[:], afd2[:], wf0[:, c : c + 1])
                    nc.scalar.mul(M11[:], afd2[:], wf1[:, c : c + 1])
                    nc.vector.tensor_tensor(M02p[:], M00[:].bitcast(F32), XpD, ALU.mult)
                    nc.vector.tensor_tensor(M12p[:], M11[:].bitcast(F32), YpD, ALU.mult)
                    nc.vector.tensor_tensor(t0[:], afd2[:], w2pd2[:], ALU.mult)
                    nc.vector.tensor_tensor(t1[:], M02p[:].bitcast(F32), XpD, ALU.mult)
                    nc.vector.tensor_tensor(t2[:], M12p[:].bitcast(F32), YpD, ALU.mult)
                    nc.vector.tensor_tensor(t1[:], t1[:], t2[:], ALU.add)
                    nc.gpsimd.tensor_tensor(M22[:], t0[:], t1[:], ALU.add)

                    res0 = wp.tile([P, SLAB], F32, name=f"res0_{c}", tag="res0")
                    res1 = wp.tile([P, SLAB], F32, name=f"res1_{c}", tag="res1")
                    res2 = wp.tile([P, SLAB], F32, name=f"res2_{c}", tag="res2")
                    nc.scalar.activation(res0[:], XpD, AF.Identity,
                                         bias=bias0[:, c : c + 1],
                                         scale=camb[:, 0:1])
                    nc.scalar.activation(res1[:], YpD, AF.Identity,
                                         bias=bias1[:, c : c + 1],
                                         scale=camb[:, 2:3])
                    nc.scalar.activation(res2[:], d, AF.Identity,
                                         bias=bias2[:, c : c + 1])

                    g0 = wp.tile([P, SLAB], F32R, name=f"g0_{c}", tag="g0")
                    g1 = wp.tile([P, SLAB], F32R, name=f"g1_{c}", tag="g1")
                    g2n = wp.tile([P, SLAB], F32R, name=f"g2n_{c}", tag="g2n")
                    t3 = wp.tile([P, SLAB], F32, name=f"t3_{c}", tag="t3")
                    t4 = wp.tile([P, SLAB], F32, name=f"t4_{c}", tag="t4")
                    t5 = wp.tile([P, SLAB], F32, name=f"t5_{c}", tag="t5")
                    nc.vector.tensor_tensor(t3[:], d, res0[:], ALU.mult)
                    nc.vector.tensor_tensor(t4[:], d, res1[:], ALU.mult)
                    nc.scalar.mul(g0[:], t3[:], camb[:, 0:1])
                    nc.scalar.mul(g1[:], t4[:], camb[:, 2:3])
                    nc.gpsimd.tensor_tensor(t3[:], XpD, g0[:].bitcast(F32), ALU.mult)
                    nc.gpsimd.tensor_tensor(t4[:], YpD, g1[:].bitcast(F32), ALU.mult)
                    nc.gpsimd.tensor_tensor(t5[:], d2, res2[:], ALU.mult)
                    nc.gpsimd.tensor_tensor(t3[:], t3[:], t4[:], ALU.add)
                    nc.gpsimd.tensor_tensor(g2n[:], t3[:], t5[:], ALU.add)

                    srcs = [M00, M11, M02p, M12p, M22, g0, g1, g2n]
                    polc = pol[:, c * NT : (c + 1) * NT]
                    for m, src in enumerate(srcs):
                        nc.tensor.matmul(
                            acc_ps[m // 4][0:NT, (m % 4) * SLAB : (m % 4 + 1) * SLAB],
                            polc, src[:],
                            start=(c == 0 and m % 2 == 0),
                            stop=(c == KC - 1 and m % 2 == 1))

                for c in range(4):
                    pass_a(c)
                batch(0)
                for c in range(4, KC):
                    pass_a(c)
                for c in range(4):
                    pass_b(c)
                batch(1)
                for c in range(4, KC):
                    pass_b(c)

            # ---------------- entry assembly via matmul -----------------
            with tc.tile_pool(name="post", bufs=2) as qp, \
                 tc.tile_pool(name="post_ps", bufs=2, space="PSUM") as qps:
                acc_sb0 = qp.tile([32, 4 * SLAB], F32R)
                acc_sb1 = qp.tile([32, 4 * SLAB], F32R)
                acc_sb = [acc_sb0, acc_sb1]
                acc_lo0 = qp.tile([32, 4 * SLAB], F32R)
                acc_lo1 = qp.tile([32, 4 * SLAB], F32R)
                acc_lo = [acc_lo0, acc_lo1]
                nc.scalar.copy(acc_sb0[0:NT, :], acc_ps0[0:NT, :])
                nc.scalar.copy(acc_sb1[0:NT, :], acc_ps1[0:NT, :])
                nc.vector.tensor_tensor(acc_lo0[0:NT, :], acc_ps0[0:NT, :],
                                        acc_sb0[0:NT, :].bitcast(F32), ALU.subtract)
                nc.vector.tensor_tensor(acc_lo1[0:NT, :], acc_ps1[0:NT, :],
                                        acc_sb1[0:NT, :].bitcast(F32), ALU.subtract)
                hent_ps = qps.tile([NE, SLAB], F32)
                for m in range(NM):
                    csl = slice((m % 4) * SLAB, (m % 4 + 1) * SLAB)
                    nc.tensor.matmul(
                        hent_ps[:], cmbt[0:NT, m * NE : (m + 1) * NE],
                        acc_sb[m // 4][0:NT, csl],
                        start=(m == 0), stop=False)
                    nc.tensor.matmul(
                        hent_ps[:], cmbt[0:NT, m * NE : (m + 1) * NE],
                        acc_lo[m // 4][0:NT, csl],
                        start=False, stop=(m == NM - 1))
                hent = qp.tile([NE, SLAB], F32)
                nc.scalar.copy(hent[:], hent_ps[:])

                # transpose to [anchor, entry]; both halves side by side
                hb = qp.tile([P, 2 * NE], F32)  # ih-major: [0:48]=ih0, [48:96]=ih1
                for ih in range(2):
                    ht_ps = qps.tile([P, NE], F32)
                    nc.tensor.transpose(ht_ps[:], hent[:, ih * P : (ih + 1) * P],
                                        ident[0:NE, 0:NE])
                    nc.scalar.copy(hb[:, ih * NE : (ih + 1) * NE], ht_ps[:])

                # ---------------- Gauss-Jordan (both halves packed) --------
                # view cols as (ih, e) with e = r*7+c over 42 used entries
                def hbv(sl):
                    return hb[:].rearrange("p (i e) -> p i e", i=2)[:, :, sl]
                piv = qp.tile([P, 2], F32)
                f12 = qp.tile([P, 12], F32)
                upd = qp.tile([P, 84], F32)
                f12v = f12[:].rearrange("p (i r) -> p i r", i=2)
                updv = upd[:].rearrange("p (i r c) -> p i r c", r=6, c=7)
                for j in range(6):
                    nc.vector.reciprocal(piv[:], hb[:, 8 * j : 2 * NE : NE])
                    nc.vector.tensor_tensor(
                        f12v, hbv(slice(j, 42, 7)),
                        piv[:].to_broadcast((P, 2, 6)), ALU.mult)
                    nc.vector.memset(f12[:, j : 12 : 6], 0.0)
                    nc.vector.tensor_tensor(
                        updv, f12v.to_broadcast((P, 2, 6, 7)),
                        hbv(slice(7 * j, 7 * j + 7)).unsqueeze(2).to_broadcast((P, 2, 6, 7)),
                        ALU.mult)
                    hview = hbv(slice(0, 42)).rearrange("p i (r c) -> p i r c", c=7)
                    nc.vector.tensor_tensor(hview, hview, updv, ALU.subtract)
                dinv = qp.tile([P, 12], F32)
                delta = qp.tile([P, 12], F32)
                dinvv = dinv[:].rearrange("p (i r) -> p i r", i=2)
                deltav = delta[:].rearrange("p (i r) -> p i r", i=2)
                nc.vector.reciprocal(dinvv, hbv(slice(0, 42, 8)))
                nc.vector.tensor_tensor(deltav, hbv(slice(6, 42, 7)), dinvv, ALU.mult)

                # ---------------- exp map coefficients (packed) ------------
                wsq = qp.tile([P, 6], F32)
                th2 = qp.tile([P, 2], F32)
                th = qp.tile([P, 2], F32)
                wv = deltav[:, :, 3:6]
                wsqv = wsq[:].rearrange("p (i r) -> p i r", i=2)
                nc.vector.tensor_tensor(wsqv, wv, wv, ALU.mult)
                nc.vector.tensor_reduce(th2[:], wsqv, AX.X, ALU.add)
                nc.scalar.sqrt(th[:], th2[:])
                mask = qp.tile([P, 2], F32)
                maski = qp.tile([P, 2], mybir.dt.int32)
                safe = qp.tile([P, 2], F32)
                nc.vector.tensor_scalar(mask[:], th[:], 1e-4, None, ALU.is_lt)
                nc.vector.tensor_copy(maski[:], mask[:])
                nc.vector.tensor_tensor(safe[:], th[:], mask[:], ALU.add)
                invs = qp.tile([P, 2], F32)
                invs2 = qp.tile([P, 2], F32)
                invs3 = qp.tile([P, 2], F32)
                nc.vector.reciprocal(invs[:], safe[:])
                nc.vector.tensor_tensor(invs2[:], invs[:], invs[:], ALU.mult)
                nc.vector.tensor_tensor(invs3[:], invs2[:], invs[:], ALU.mult)
                sh = qp.tile([P, 2], F32)
                sh2 = qp.tile([P, 2], F32)
                csf = qp.tile([P, 2], F32)
                snf = qp.tile([P, 2], F32)
                nc.scalar.activation(sh[:], safe[:], AF.Sin, scale=0.5)
                nc.vector.tensor_tensor(sh2[:], sh[:], sh[:], ALU.mult)
                nc.vector.tensor_scalar(csf[:], sh2[:], -2.0, 1.0, ALU.mult, ALU.add)
                nc.vector.tensor_scalar(snf[:], sh2[:], -1.0, 1.0, ALU.mult, ALU.add)
                nc.scalar.sqrt(snf[:], snf[:])
                nc.vector.tensor_tensor(snf[:], snf[:], sh[:], ALU.mult)
                nc.vector.tensor_scalar(snf[:], snf[:], 2.0, None, ALU.mult)
                abc = qp.tile([P, 6], F32)   # col = coeff(A,B,C)*2 + ih
                abct = qp.tile([P, 6], F32)
                tmp2 = qp.tile([P, 2], F32)
                nc.vector.tensor_tensor(abc[:, 0:2], snf[:], invs[:], ALU.mult)
                nc.vector.tensor_scalar(tmp2[:], csf[:], -1.0, 1.0, ALU.mult, ALU.add)
                nc.vector.tensor_tensor(abc[:, 2:4], tmp2[:], invs2[:], ALU.mult)
                nc.vector.tensor_tensor(tmp2[:], safe[:], snf[:], ALU.subtract)
                nc.vector.tensor_tensor(abc[:, 4:6], tmp2[:], invs3[:], ALU.mult)
                nc.vector.tensor_scalar(abct[:, 0:2], th2[:], -1.0 / 6.0, 1.0, ALU.mult, ALU.add)
                nc.vector.tensor_scalar(abct[:, 2:4], th2[:], -1.0 / 24.0, 0.5, ALU.mult, ALU.add)
                nc.vector.tensor_scalar(abct[:, 4:6], th2[:], -1.0 / 120.0, 1.0 / 6.0, ALU.mult, ALU.add)
                mask6 = qp.tile([P, 6], mybir.dt.int32)
                nc.vector.tensor_copy(
                    mask6[:].rearrange("p (a i) -> p a i", i=2),
                    maski[:].unsqueeze(1).to_broadcast((P, 3, 2)))
                nc.vector.copy_predicated(abc[:], mask6[:], abct[:])

                # ------- packed both-half R/V, translation, compose --------
                # layouts: delta [P,12] (ih-major r), wsq [P,6] (ih,r),
                # abc [P,6] (coeff*2+ih), th2 [P,2]
                def iv(tile_ap, n):
                    return tile_ap.rearrange("p (i e) -> p i e", i=n)
                wb = deltav[:, :, 3:6]          # (P,2,3)
                vb = deltav[:, :, 0:3]
                u3 = qp.tile([P, 6], F32)       # (ih, r)
                u3v = iv(u3[:], 2)
                nc.vector.tensor_tensor(
                    u3v, wsqv, th2[:].unsqueeze(2).to_broadcast((P, 2, 3)),
                    ALU.subtract)
                Aw = qp.tile([P, 6], F32)
                Bw = qp.tile([P, 6], F32)
                Cw = qp.tile([P, 6], F32)
                dB = qp.tile([P, 6], F32)
                dC = qp.tile([P, 6], F32)
                nc.vector.tensor_tensor(
                    iv(Aw[:], 2), wb,
                    abc[:, 0:2].unsqueeze(2).to_broadcast((P, 2, 3)), ALU.mult)
                nc.vector.tensor_tensor(
                    iv(Bw[:], 2), wb,
                    abc[:, 2:4].unsqueeze(2).to_broadcast((P, 2, 3)), ALU.mult)
                nc.vector.tensor_tensor(
                    iv(Cw[:], 2), wb,
                    abc[:, 4:6].unsqueeze(2).to_broadcast((P, 2, 3)), ALU.mult)
                nc.vector.tensor_tensor(
                    iv(dB[:], 2), u3v,
                    abc[:, 2:4].unsqueeze(2).to_broadcast((P, 2, 3)), ALU.mult)
                nc.vector.tensor_tensor(
                    iv(dC[:], 2), u3v,
                    abc[:, 4:6].unsqueeze(2).to_broadcast((P, 2, 3)), ALU.mult)
                # w components for both halves: delta cols 3+r and 9+r
                def wcol(r):
                    return delta[:, 3 + r : 12 : 6]
                qb = qp.tile([P, 6], F32)   # q01,q02,q12 x (2 ih): col=q*2+ih
                cb = qp.tile([P, 6], F32)
                nc.vector.tensor_tensor(qb[:, 0:2], Bw[:, 0:6:3], wcol(1), ALU.mult)
                nc.vector.tensor_tensor(qb[:, 2:4], Bw[:, 0:6:3], wcol(2), ALU.mult)
                nc.vector.tensor_tensor(qb[:, 4:6], Bw[:, 1:6:3], wcol(2), ALU.mult)
                nc.vector.tensor_tensor(cb[:, 0:2], Cw[:, 0:6:3], wcol(1), ALU.mult)
                nc.vector.tensor_tensor(cb[:, 2:4], Cw[:, 0:6:3], wcol(2), ALU.mult)
                nc.vector.tensor_tensor(cb[:, 4:6], Cw[:, 1:6:3], wcol(2), ALU.mult)

                # Rt/Vt: [P,18], col = entry*2 + ih
                Rt = qp.tile([P, 18], F32)
                Vt = qp.tile([P, 18], F32)
                for M, hat, dgc, oc in ((Rt, Aw, dB, qb), (Vt, Bw, dC, cb)):
                    nc.vector.tensor_scalar(M[:, 0:2], dgc[:, 0:6:3], 1.0, None, ALU.add)
                    nc.vector.tensor_tensor(M[:, 2:4], oc[:, 0:2], hat[:, 2:6:3], ALU.subtract)
                    nc.vector.tensor_tensor(M[:, 4:6], oc[:, 2:4], hat[:, 1:6:3], ALU.add)
                    nc.vector.tensor_tensor(M[:, 6:8], oc[:, 0:2], hat[:, 2:6:3], ALU.add)
                    nc.vector.tensor_scalar(M[:, 8:10], dgc[:, 1:6:3], 1.0, None, ALU.add)
                    nc.vector.tensor_tensor(M[:, 10:12], oc[:, 4:6], hat[:, 0:6:3], ALU.subtract)
                    nc.vector.tensor_tensor(M[:, 12:14], oc[:, 2:4], hat[:, 1:6:3], ALU.subtract)
                    nc.vector.tensor_tensor(M[:, 14:16], oc[:, 4:6], hat[:, 0:6:3], ALU.add)
                    nc.vector.tensor_scalar(M[:, 16:18], dgc[:, 2:6:3], 1.0, None, ALU.add)

                # translation t = V @ v  -> tvb [P,6] col = r*2 + ih
                tvb = qp.tile([P, 6], F32)
                trow = qp.tile([P, 6], F32)
                for r in range(3):
                    vrow = Vt[:, 6 * r : 6 * r + 6].rearrange("p (c i) -> p i c", i=2)
                    nc.vector.tensor_tensor(iv(trow[:], 2), vrow, vb, ALU.mult)
                    nc.vector.tensor_reduce(tvb[:, 2 * r : 2 * r + 2], iv(trow[:], 2),
                                            AX.X, ALU.add)

                # compose out = dT @ Tmat, packed [P, 32] (ih-major)
                tmib = qp.tile([P, 32], F32)
                nc.vector.tensor_copy(tmib[:, 0:16], tmi0[:])
                nc.vector.tensor_copy(tmib[:, 16:32], tmi1[:])
                tmv = iv(tmib[:], 2)            # (P,2,16)
                Ob = qp.tile([P, 32], F32)
                Obv = iv(Ob[:], 2)
                oac = qp.tile([P, 8], F32)
                oacv = iv(oac[:], 2)
                for r in range(3):
                    orow = Obv[:, :, 4 * r : 4 * r + 4]
                    nc.vector.tensor_tensor(
                        orow, tmv[:, :, 0:4],
                        Rt[:, 6 * r : 6 * r + 2].unsqueeze(2).to_broadcast((P, 2, 4)),
                        ALU.mult)
                    nc.vector.tensor_tensor(
                        oacv, tmv[:, :, 4:8],
                        Rt[:, 6 * r + 2 : 6 * r + 4].unsqueeze(2).to_broadcast((P, 2, 4)),
                        ALU.mult)
                    nc.vector.tensor_tensor(orow, orow, oacv, ALU.add)
                    nc.vector.tensor_tensor(
                        oacv, tmv[:, :, 8:12],
                        Rt[:, 6 * r + 4 : 6 * r + 6].unsqueeze(2).to_broadcast((P, 2, 4)),
                        ALU.mult)
                    nc.vector.tensor_tensor(orow, orow, oacv, ALU.add)
                    nc.vector.tensor_tensor(
                        oacv, tmv[:, :, 12:16],
                        tvb[:, 2 * r : 2 * r + 2].unsqueeze(2).to_broadcast((P, 2, 4)),
                        ALU.mult)
                    nc.vector.tensor_tensor(orow, orow, oacv, ALU.add)
                nc.vector.tensor_copy(Obv[:, :, 12:16], tmv[:, :, 12:16])
                nc.sync.dma_start(out_d[0:P, :], Ob[:, 0:16])
                nc.sync.dma_start(out_d[P : 2 * P, :], Ob[:, 16:32])

    nc.compile()
    return nc


def prep_inputs(embeddings, revisions, weights, depth, pix_T_camXs, Tmat):
    f = np.float32
    emb = np.ascontiguousarray(embeddings, dtype=f).reshape(B, C, N)
    rev = np.ascontiguousarray(revisions, dtype=f).reshape(B, 3, N)
    wgt = np.ascontiguousarray(weights, dtype=f).reshape(B, 3, N)
    dep = np.ascontiguousarray(depth, dtype=f).reshape(B, N)
    pix = np.ascontiguousarray(pix_T_camXs, dtype=f)
    tm = np.ascontiguousarray(Tmat, dtype=f).reshape(B, N, 16)

    ys, xs = np.meshgrid(np.arange(H, dtype=f), np.arange(W, dtype=f), indexing="ij")
    u = xs.reshape(-1)
    v = ys.reshape(-1)

    def kmaj(a):
        a = a.reshape(KC, P, -1).transpose(1, 0, 2)
        return np.ascontiguousarray(a.reshape(P, -1), dtype=f)

    uk = kmaj(u)
    vk = kmaj(v)
    cmb = combo_matrices()
    in_maps = []
    for core in range(NCORES):
        b = core // 4
        s0 = (core % 4) * SLAB
        fx, fy, x0, y0 = pix[b, 0, 0], pix[b, 1, 1], pix[b, 0, 2], pix[b, 1, 2]
        tms = tm[b][s0 : s0 + SLAB]       # [256, 16]
        # joint moving operand [21, 1024]:
        # cols 0:256 Xp, 256:512 Yp, 512:768 Zp, 768:1024 s (affinity arg)
        # rows 0:16 pair with -2e_k; 16:19 with x,y,z; 19 with ones; 20 with ek2
        rhsj = np.zeros((21, 4 * SLAB), f)
        for p in range(3):
            for q in range(3):
                rhsj[C + q, p * SLAB : (p + 1) * SLAB] = tms[:, 4 * p + q]
            rhsj[C + 3, p * SLAB : (p + 1) * SLAB] = tms[:, 4 * p + 3]
        rhsj[0:C, 3 * SLAB : 4 * SLAB] = emb[b][:, s0 : s0 + SLAB]
        rhsj[C + 4, 3 * SLAB : 4 * SLAB] = 1.0
        # row C+3 (ones multiplier) cols 768:1024 = ei2, filled on device
        kblob = np.concatenate(
            [kmaj(dep[b]), uk, vk, kmaj(tm[b]), kmaj(rev[b].T), kmaj(wgt[b].T)], 1)
        in_maps.append({
            "kblob": np.ascontiguousarray(kblob),
            "ek": emb[b],
            "ei": np.ascontiguousarray(emb[b][:, s0 : s0 + SLAB]),
            "uvzz": np.ascontiguousarray(np.stack([u, v], 0)),
            "zz2": np.ascontiguousarray(np.stack([dep[b], dep[b]], 0)),
            "zo": np.ascontiguousarray(np.stack([dep[b], np.ones(N, f)], 0)),
            "cam": np.array([[fx, x0, fy, y0]], dtype=f),
            "cam2": np.array([[x0, fx], [y0, fy]], dtype=f),
            "rhsj": rhsj,
            "cmbt": cmb,
            "tm_i": np.ascontiguousarray(tms),
        })
    return in_maps


def gather_output(results):
    full = np.empty((B, N, 16), dtype=np.float32)
    for core in range(NCORES):
        b = core // 4
        s0 = (core % 4) * SLAB
        full[b, s0 : s0 + SLAB] = results[core]["out"]
    return full.reshape(B, H, W, 4, 4)


_NC_CACHE = {}


def kernel(**inputs):
    if "nc" not in _NC_CACHE:
        _NC_CACHE["nc"] = build_nc()
    nc = _NC_CACHE["nc"]
    in_maps = prep_inputs(**inputs)
    res = run_bass_kernel_spmd(nc, in_maps, core_ids=list(range(NCORES)))
    return gather_output(res.results)

